# revision 19
# baseline (speedup 1.0000x reference)
"""Correlation-layer cosine-similarity kernel for Trainium2 (8 NeuronCores).

Problem: x1, x2: [B=4, C=256, H=128, W=256] fp32.
out[b, d, h, w] = cos-sim over C of (x1[b,:,h,w], x2_padded[b,:,h,w+d]), d in 0..40.

Sharding: core i handles batch b = i//2 and H-half hh = i%2 (64 rows).

v3 design (v1 was sequencer/DMA-count bound; v2 batched DMAs per 8-row block):
- squares and cross-kc sums via scalar_tensor_tensor (4x DVE mode).
- norms computed TRANSPOSED ([w on partitions]) via ldweights matmuls against a
  ones column: sqrt/recip run lane-parallel, 1/n1 comes out as ready-to-use
  per-partition scalars, and the 1/n2 rows for the DRAM bounce are produced by
  a PE transpose so the row write is 16 fat descriptors instead of 2048 2-byte
  ones.
- x2's zero pad is never materialized: the Gram's pad columns are zeroed in
  the fp16 cover and the pad entries of 1/n2 are constants written once.
- Gram normalized by 1/n1 straight out of PSUM (Act/DVE split), bounced
  through DRAM in fp16, read back with a skewed AP that extracts the 41
  diagonals; 1/n2 rows come back with the same skew.
- fp16 band math, PE transpose per row to [(mc,d), w], copies alternate
  DVE/Act, two output DMAs per block, triple-buffered DRAM scratch.
"""

import numpy as np

B, C, H, W = 4, 256, 128, 256
D = 41           # displacements 0..40
HC = 64          # H rows per core
PAD = 40
W2 = W + PAD     # 296
COVER = 168      # gram cover columns per 128-row block
HB = 8           # h rows per block
NB = HC // HB    # 8 blocks
NSC = 3          # DRAM scratch depth

_cache = {}


def _build_nc():
    import concourse.bass as bass
    import concourse.tile as tile
    from concourse import bacc, mybir
    from concourse.masks import make_identity

    f32 = mybir.dt.float32
    bf16 = mybir.dt.bfloat16
    f16 = mybir.dt.float16
    Alu = mybir.AluOpType

    nc = bacc.Bacc(trn_type="TRN2")
    x1s = nc.dram_tensor("x1s", [C, HC, W], f32, kind="ExternalInput")
    x2s = nc.dram_tensor("x2s", [C, HC, W], f32, kind="ExternalInput")
    outs = nc.dram_tensor("outs", [D, HC, W], f32, kind="ExternalOutput")
    gdd = [nc.dram_tensor(f"gd{k}", [HB, 128, 2 * COVER], f16, kind="Internal")
           for k in range(NSC)]
    ndd = [nc.dram_tensor(f"nd{k}", [HB, W2], f16, kind="Internal")
           for k in range(NSC)]

    with tile.TileContext(nc) as tc:
        with (
            tc.tile_pool(name="const", bufs=1) as constp,
            tc.tile_pool(name="io", bufs=3) as io,
            tc.tile_pool(name="sqp", bufs=3) as sqp,
            tc.tile_pool(name="sp", bufs=3) as sp,
            tc.tile_pool(name="small", bufs=3) as small,
            tc.tile_pool(name="gsbp", bufs=3) as gsbp,
            tc.tile_pool(name="bp", bufs=3) as bp,
            tc.tile_pool(name="outp", bufs=3) as outp,
            tc.tile_pool(name="gp", bufs=3, space="PSUM") as gp,
            tc.tile_pool(name="npp", bufs=2, space="PSUM") as npp,
            tc.tile_pool(name="tpp", bufs=2, space="PSUM") as tpp,
        ):
            onesb = constp.tile([128, 1], bf16)
            nc.vector.memset(onesb, 1.0)
            epsb = constp.tile([128, 1], f32)
            nc.vector.memset(epsb, 1e-6)
            identh = constp.tile([128, 128], f16)
            make_identity(nc, identh)
            # constant 1/n2 for the zero-pad columns (band there is exactly 0,
            # so any finite value preserves ref's 0 output)
            padc = constp.tile([HB, PAD], f16)
            nc.vector.memset(padc, 1.0)
            for k in range(NSC):
                dst_p = bass.AP(tensor=ndd[k], offset=W,
                                ap=[[W2, HB], [1, PAD]])
                nc.sync.dma_start(out=dst_p, in_=padc)

            for hb in range(NB):
                gd = gdd[hb % NSC]
                nd = ndd[hb % NSC]

                # ---- input load (cast f32->bf16), one DMA per tensor
                x1b = io.tile([128, 2, HB, W], bf16, tag="x1b")
                x2b = io.tile([128, 2, HB, W], bf16, tag="x2b")
                for dst, srct in ((x1b, x1s), (x2b, x2s)):
                    src = bass.AP(tensor=srct, offset=hb * HB * W,
                                  ap=[[HC * W, 128], [128 * HC * W, 2], [1, HB * W]])
                    nc.gpsimd.dma_start(out=dst.rearrange("p a b w -> p a (b w)"),
                                        in_=src)

                # ---- squares (split Act/DVE) and kc-sums (DVE, 2x bf16)
                sq1 = sqp.tile([128, 2, HB, W], bf16, tag="sq1")
                nc.scalar.square(sq1[:, 0], x1b[:, 0])
                nc.vector.tensor_mul(sq1[:, 1], x1b[:, 1], x1b[:, 1])
                sq2 = sqp.tile([128, 2, HB, W], bf16, tag="sq2")
                nc.vector.tensor_mul(sq2, x2b, x2b)
                s1 = sp.tile([128, HB, W], bf16, tag="s1")
                nc.vector.tensor_add(s1, sq1[:, 0], sq1[:, 1])
                s2 = sp.tile([128, HB, W], bf16, tag="s2")
                nc.vector.tensor_add(s2, sq2[:, 0], sq2[:, 1])

                # ---- transposed norm sums: pT[w, k, j];
                #      k: 0,1 = n1 w-blocks; 2,3 = n2 w2-blocks
                pT = npp.tile([128, 4, HB], f32, tag="pT")
                for j in range(HB):
                    for mc in range(2):
                        nc.tensor.matmul(pT[:, mc, j:j + 1],
                                         s1[:, j, 128 * mc:128 * (mc + 1)], onesb,
                                         start=True, stop=True)
                        nc.tensor.matmul(pT[:, 2 + mc, j:j + 1],
                                         s2[:, j, 128 * mc:128 * (mc + 1)], onesb,
                                         start=True, stop=True)

                # ---- 1/sqrt(nsq + eps): Act sqrt then DVE reciprocal
                sn = small.tile([128, 4, HB], f32, tag="sn")
                nc.scalar.activation(out=sn, in_=pT,
                                     func=mybir.ActivationFunctionType.Sqrt,
                                     bias=epsb, scale=1.0)
                vinv = small.tile([128, 4, HB], f32, tag="vinv")
                nc.vector.reciprocal_approx_fast(out=vinv, in_=sn)

                # ---- fp16 1/n1 columns for the post-skew normalize
                n1t = small.tile([128, 2, HB], f16, tag="n1t")
                nc.scalar.copy(n1t, vinv[:, 0:2, :])

                # ---- 1/n2 rows to DRAM via PE transpose (fat descriptors)
                n2t = small.tile([128, 2, HB], f16, tag="n2t")
                nc.scalar.copy(n2t, vinv[:, 2:4, :])
                n2tp = tpp.tile([2 * HB, 128], f16, tag="n2tp", bufs=1)
                nc.tensor.transpose(n2tp, n2t, identh)
                rows = small.tile([2 * HB, 128], f16, tag="rows")
                nc.scalar.copy(rows, n2tp)
                dst_n = bass.AP(tensor=nd, offset=0,
                                ap=[[128, 2], [W2, HB], [1, 128]])
                nc.sync.dma_start(out=dst_n, in_=rows)

                # ---- gram + 1/n1 normalize into fp16 cover
                gsb = gsbp.tile([128, HB, 2, COVER], f16, tag="gsb")
                # pad columns of the mc=1 cover (w2 >= 256) are exactly zero
                nc.gpsimd.memset(gsb[:, :, 1, 128:COVER], 0.0)
                for j in range(HB):
                    G = gp.tile([128, 2, COVER], f32, tag="g")
                    for kc in range(2):
                        nc.tensor.matmul(G[:, 0, :], x1b[:, kc, j, 0:128],
                                         x2b[:, kc, j, 0:COVER],
                                         start=(kc == 0), stop=(kc == 1))
                    for kc in range(2):
                        nc.tensor.matmul(G[:, 1, 0:128], x1b[:, kc, j, 128:256],
                                         x2b[:, kc, j, 128:W],
                                         start=(kc == 0), stop=(kc == 1))
                    nc.scalar.copy(gsb[:, j, 0, :], G[:, 0, :])
                    if j % 2 == 0:
                        nc.scalar.copy(gsb[:, j, 1, 0:128], G[:, 1, 0:128])
                    else:
                        nc.vector.tensor_copy(gsb[:, j, 1, 0:128], G[:, 1, 0:128])

                # ---- bounce: cover to DRAM, skewed reads back
                dst_g = bass.AP(tensor=gd, offset=0,
                                ap=[[2 * COVER, 128], [128 * 2 * COVER, HB],
                                    [1, 2 * COVER]])
                nc.sync.dma_start(out=dst_g,
                                  in_=gsb.rearrange("p h m t -> p h (m t)"))
                band = bp.tile([128, HB, 2, D], f16, tag="band")
                n2sk = bp.tile([128, HB, 2, D], f16, tag="n2sk")
                for mc in range(2):
                    src_band = bass.AP(tensor=gd, offset=mc * COVER,
                                       ap=[[2 * COVER + 1, 128],
                                           [128 * 2 * COVER, HB], [1, D]])
                    nc.scalar.dma_start(out=band[:, :, mc, :], in_=src_band)
                    src_n2 = bass.AP(tensor=nd, offset=mc * 128,
                                     ap=[[1, 128], [W2, HB], [1, D]])
                    nc.sync.dma_start(out=n2sk[:, :, mc, :], in_=src_n2)

                # ---- final normalize + transpose + out
                nprod = bp.tile([128, HB, 2, D], f16, tag="nprod")
                n1bc = n1t.transpose([0, 2, 1]).unsqueeze(3).broadcast_to(
                    [128, HB, 2, D])
                nc.gpsimd.tensor_mul(nprod, n2sk, n1bc)
                bn = bp.tile([128, HB, 2, D], f16, tag="bn")
                nc.gpsimd.tensor_mul(bn, band, nprod)
                out_sb = outp.tile([2 * D, HB, 128], f32, tag="out_sb")
                for j in range(HB):
                    tp = tpp.tile([2 * D, 128], f16, tag="tp")
                    nc.tensor.transpose(tp, bn[:, j], identh)
                    if j % 2 == 0:
                        nc.vector.tensor_copy(out_sb[:, j, :], tp)
                    else:
                        nc.scalar.copy(out_sb[:, j, :], tp)
                for mc in range(2):
                    dst_o = bass.AP(tensor=outs, offset=hb * HB * W + mc * 128,
                                    ap=[[HC * W, D], [W, HB], [1, 128]])
                    nc.sync.dma_start(out=dst_o, in_=out_sb[D * mc:D * (mc + 1)])

    nc.finalize()
    return nc


def _in_maps(x_1, x_2):
    maps = []
    for i in range(8):
        b, hh = i // 2, i % 2
        sl = slice(hh * HC, (hh + 1) * HC)
        maps.append({
            "x1s": np.ascontiguousarray(x_1[b, :, sl, :]),
            "x2s": np.ascontiguousarray(x_2[b, :, sl, :]),
        })
    return maps


def kernel(x_1: np.ndarray, x_2: np.ndarray) -> np.ndarray:
    from concourse.bass_utils import run_bass_kernel_spmd

    if "nc" not in _cache:
        _cache["nc"] = _build_nc()
    nc = _cache["nc"]

    x_1 = np.asarray(x_1, dtype=np.float32)
    x_2 = np.asarray(x_2, dtype=np.float32)
    res = run_bass_kernel_spmd(nc, _in_maps(x_1, x_2), core_ids=list(range(8)))
    out = np.empty((B, D, H, W), dtype=np.float32)
    for i in range(8):
        b, hh = i // 2, i % 2
        out[b, :, hh * HC:(hh + 1) * HC, :] = res.results[i]["outs"]
    return out


# revision 38
# speedup vs baseline: 55.8128x; 55.8128x over previous
"""Correlation-layer cosine-similarity kernel for Trainium2 (8 NeuronCores).

Problem: x1, x2: [B=4, C=256, H=128, W=256] fp32.
out[b, d, h, w] = cos-sim over C of (x1[b,:,h,w], x2_padded[b,:,h,w+d]), d in 0..40.

Sharding: core i handles batch b = i//2 and H-half hh = i%2 (64 rows).

Design (the v1 baseline was sequencer/DMA-count bound at ~383 us in the cost
model; this version sims at ~110 us, close to its ~84 us DMA floor):
- h-rows processed in blocks of HB=8; DMAs batched per block (~9 HWDGE + 2
  SWDGE per block vs ~56 in v1), issue spread across SP/gpsimd sequencers.
- per h: Gram G[w, w2] = x1^T x2 over C via bf16 matmuls (n=168 covers, the
  minimal band cover); squares/kc-sums on DVE+Act feed TRANSPOSED norm sums
  ([w on partitions], ldweights matmuls against a ones column) so sqrt/recip
  run lane-parallel and 1/n1 needs no row->column conversion.
- x2's zero pad is never materialized: the Gram cover's pad columns and the
  pad entries of 1/n2 are constants written to the scratch tensors once.
- the 41 diagonals are extracted by bouncing the cover through DRAM in fp16
  and reading back with a skewed AP (stride 337 walks the diagonal); 1/n2
  rows bounce the same way (written via a PE transpose so the row write is
  16 fat descriptors instead of 2048 2-byte ones).
- normalization off the critical path: bn = band * (n2sk * bcast(1/n1)) in
  fp16 (free-dim stride-0 broadcast); PE transposes [w,(mc,d)] -> [(mc,d),w]
  per row; PSUM->SBUF copies alternate DVE/Act; 2 output DMAs per block;
  triple-buffered DRAM scratch decouples consecutive blocks' bounces.
"""

import numpy as np

B, C, H, W = 4, 256, 128, 256
D = 41           # displacements 0..40
HC = 64          # H rows per core
PAD = 40
W2 = W + PAD     # 296
COVER = 168      # gram cover columns per 128-row block
HB = 8           # h rows per block
NB = HC // HB    # 8 blocks
NSC = 3          # DRAM scratch depth

_cache = {}


def _build_nc(reps=1):
    import concourse.bass as bass
    import concourse.tile as tile
    from concourse import bacc, mybir
    from concourse.masks import make_identity

    f32 = mybir.dt.float32
    bf16 = mybir.dt.bfloat16
    f16 = mybir.dt.float16
    Alu = mybir.AluOpType

    nc = bacc.Bacc(trn_type="TRN2")
    x1s = nc.dram_tensor("x1s", [C, HC, W], f32, kind="ExternalInput")
    x2s = nc.dram_tensor("x2s", [C, HC, W], f32, kind="ExternalInput")
    outs = nc.dram_tensor("outs", [D, HC, W], f32, kind="ExternalOutput")
    gdd = [nc.dram_tensor(f"gd{k}", [HB, 128, 2 * COVER], f16, kind="Internal")
           for k in range(NSC)]
    ndd = [nc.dram_tensor(f"nd{k}", [HB, W2], f16, kind="Internal")
           for k in range(NSC)]

    with tile.TileContext(nc) as tc:
        with (
            tc.tile_pool(name="const", bufs=1) as constp,
            tc.tile_pool(name="io", bufs=3) as io,
            tc.tile_pool(name="sqp", bufs=3) as sqp,
            tc.tile_pool(name="sp", bufs=3) as sp,
            tc.tile_pool(name="small", bufs=3) as small,
            tc.tile_pool(name="gsbp", bufs=3) as gsbp,
            tc.tile_pool(name="bp", bufs=3) as bp,
            tc.tile_pool(name="outp", bufs=3) as outp,
            tc.tile_pool(name="gp", bufs=2, space="PSUM") as gp,
            tc.tile_pool(name="npp", bufs=2, space="PSUM") as npp,
            tc.tile_pool(name="tpp", bufs=2, space="PSUM") as tpp,
        ):
            onesb = constp.tile([128, 1], bf16)
            nc.vector.memset(onesb, 1.0)
            epsb = constp.tile([128, 1], f32)
            nc.vector.memset(epsb, 1e-6)
            identh = constp.tile([128, 128], f16)
            make_identity(nc, identh)
            # constant 1/n2 for the zero-pad columns (band there is exactly 0,
            # so any finite value preserves ref's 0 output)
            padc = constp.tile([HB, PAD], f16)
            nc.vector.memset(padc, 1.0)
            zpad = constp.tile([128, HB, PAD], f16)
            nc.vector.memset(zpad, 0.0)
            for k in range(NSC):
                dst_p = bass.AP(tensor=ndd[k], offset=W,
                                ap=[[W2, HB], [1, PAD]])
                nc.sync.dma_start(out=dst_p, in_=padc)
                # gram-cover pad columns (w2 >= 256) are always exactly zero
                dst_z = bass.AP(tensor=gdd[k], offset=2 * COVER - PAD,
                                ap=[[2 * COVER, 128], [128 * 2 * COVER, HB],
                                    [1, PAD]])
                nc.sync.dma_start(out=dst_z, in_=zpad)

            blocks = [(k * HB, HB) for k in range(NB)]
            for it, (h0, bs) in enumerate(blocks * reps):
                gd = gdd[it % NSC]
                nd = ndd[it % NSC]

                # ---- input load (cast f32->bf16), one DMA per tensor
                x1b = io.tile([128, 2, bs, W], bf16, tag="x1b")
                x2b = io.tile([128, 2, bs, W], bf16, tag="x2b")
                for dst, srct in ((x1b, x1s), (x2b, x2s)):
                    src = bass.AP(tensor=srct, offset=h0 * W,
                                  ap=[[HC * W, 128], [128 * HC * W, 2], [1, bs * W]])
                    nc.gpsimd.dma_start(out=dst.rearrange("p a b w -> p a (b w)"),
                                        in_=src)

                # ---- squares (split Act/DVE) and kc-sums (DVE, 2x bf16)
                sq1 = sqp.tile([128, 2, bs, W], bf16, tag="sq1")
                nc.scalar.square(sq1[:, 0], x1b[:, 0])
                nc.vector.tensor_mul(sq1[:, 1], x1b[:, 1], x1b[:, 1])
                sq2 = sqp.tile([128, 2, bs, W], bf16, tag="sq2")
                nc.vector.tensor_mul(sq2, x2b, x2b)
                s1 = sp.tile([128, bs, W], bf16, tag="s1")
                nc.vector.tensor_add(s1, sq1[:, 0], sq1[:, 1])
                s2 = sp.tile([128, bs, W], bf16, tag="s2")
                nc.vector.tensor_add(s2, sq2[:, 0], sq2[:, 1])

                # ---- transposed norm sums: pT[w, k, j];
                #      k: 0,1 = n1 w-blocks; 2,3 = n2 w2-blocks
                pT = npp.tile([128, 4, bs], f32, tag="pT")
                for j in range(bs):
                    for mc in range(2):
                        nc.tensor.matmul(pT[:, mc, j:j + 1],
                                         s1[:, j, 128 * mc:128 * (mc + 1)], onesb,
                                         start=True, stop=True)
                        nc.tensor.matmul(pT[:, 2 + mc, j:j + 1],
                                         s2[:, j, 128 * mc:128 * (mc + 1)], onesb,
                                         start=True, stop=True)

                # ---- 1/sqrt(nsq + eps): Act sqrt then DVE reciprocal
                sn = small.tile([128, 4, bs], f32, tag="sn")
                nc.scalar.activation(out=sn, in_=pT,
                                     func=mybir.ActivationFunctionType.Sqrt,
                                     bias=epsb, scale=1.0)
                vinv = small.tile([128, 4, bs], f32, tag="vinv")
                nc.vector.reciprocal_approx_fast(out=vinv, in_=sn)

                # ---- fp16 1/n1 columns for the post-skew normalize
                n1t = small.tile([128, 2, bs], f16, tag="n1t")
                nc.scalar.copy(n1t, vinv[:, 0:2, :])

                # ---- 1/n2 rows to DRAM via PE transpose (fat descriptors)
                n2t = small.tile([128, 2, bs], f16, tag="n2t")
                nc.scalar.copy(n2t, vinv[:, 2:4, :])
                n2tp = tpp.tile([2 * bs, 128], f16, tag="n2tp", bufs=1)
                nc.tensor.transpose(n2tp, n2t, identh)
                rows = small.tile([2 * bs, 128], f16, tag="rows")
                nc.scalar.copy(rows, n2tp)
                dst_n = bass.AP(tensor=nd, offset=0,
                                ap=[[128, 2], [W2, bs], [1, 128]])
                nc.sync.dma_start(out=dst_n, in_=rows)

                # ---- gram + 1/n1 normalize into fp16 cover
                gsb = gsbp.tile([128, bs, W2], f16, tag="gsb")
                for j in range(bs):
                    G = gp.tile([128, 2, COVER], f32, tag="g")
                    for kc in range(2):
                        nc.tensor.matmul(G[:, 0, :], x1b[:, kc, j, 0:128],
                                         x2b[:, kc, j, 0:COVER],
                                         start=(kc == 0), stop=(kc == 1))
                    for kc in range(2):
                        nc.tensor.matmul(G[:, 1, 0:128], x1b[:, kc, j, 128:256],
                                         x2b[:, kc, j, 128:W],
                                         start=(kc == 0), stop=(kc == 1))
                    nc.scalar.copy(gsb[:, j, 0:COVER], G[:, 0, :])
                    if j % 2 == 0:
                        nc.scalar.copy(gsb[:, j, COVER:W2], G[:, 1, 0:128])
                    else:
                        nc.vector.tensor_copy(gsb[:, j, COVER:W2], G[:, 1, 0:128])

                # ---- n2 skew read (early; off the drain path)
                n2sk = bp.tile([128, bs, 2, D], f16, tag="n2sk")
                for mc in range(2):
                    src_n2 = bass.AP(tensor=nd, offset=mc * 128,
                                     ap=[[1, 128], [W2, bs], [1, D]])
                    nc.sync.dma_start(out=n2sk[:, :, mc, :], in_=src_n2)
                nprod = bp.tile([128, bs, 2, D], f16, tag="nprod")
                n1bc = n1t.transpose([0, 2, 1]).unsqueeze(3).broadcast_to(
                    [128, bs, 2, D])
                nc.gpsimd.tensor_mul(nprod, n2sk, n1bc)

                # ---- bounce: cover to DRAM, skewed band read back
                dst_g = bass.AP(tensor=gd, offset=0,
                                ap=[[2 * COVER, 128], [128 * 2 * COVER, bs],
                                    [1, W2]])
                nc.sync.dma_start(out=dst_g, in_=gsb)
                band = bp.tile([128, bs, 2, D], f16, tag="band")
                for mc in range(2):
                    src_band = bass.AP(tensor=gd, offset=mc * COVER,
                                       ap=[[2 * COVER + 1, 128],
                                           [128 * 2 * COVER, bs], [1, D]])
                    nc.sync.dma_start(out=band[:, :, mc, :], in_=src_band)

                # ---- final normalize + transpose + out
                bn = bp.tile([128, bs, 2, D], f16, tag="bn")
                nc.vector.tensor_mul(bn, band, nprod)
                out_sb = outp.tile([2 * D, bs, 128], f32, tag="out_sb")
                for j in range(bs):
                    tp = tpp.tile([2 * D, 128], f16, tag="tp", bufs=3)
                    nc.tensor.transpose(tp, bn[:, j], identh)
                    if j % 2 == 0:
                        nc.vector.tensor_copy(out_sb[:, j, :], tp)
                    else:
                        nc.scalar.copy(out_sb[:, j, :], tp)
                for mc in range(2):
                    dst_o = bass.AP(tensor=outs, offset=h0 * W + mc * 128,
                                    ap=[[HC * W, D], [W, bs], [1, 128]])
                    nc.sync.dma_start(out=dst_o, in_=out_sb[D * mc:D * (mc + 1)])

    nc.finalize()
    return nc


def _in_maps(x_1, x_2):
    maps = []
    for i in range(8):
        b, hh = i // 2, i % 2
        sl = slice(hh * HC, (hh + 1) * HC)
        maps.append({
            "x1s": np.ascontiguousarray(x_1[b, :, sl, :]),
            "x2s": np.ascontiguousarray(x_2[b, :, sl, :]),
        })
    return maps


def kernel(x_1: np.ndarray, x_2: np.ndarray) -> np.ndarray:
    from concourse.bass_utils import run_bass_kernel_spmd

    if "nc" not in _cache:
        _cache["nc"] = _build_nc()
    nc = _cache["nc"]

    x_1 = np.asarray(x_1, dtype=np.float32)
    x_2 = np.asarray(x_2, dtype=np.float32)
    res = run_bass_kernel_spmd(nc, _in_maps(x_1, x_2), core_ids=list(range(8)))
    out = np.empty((B, D, H, W), dtype=np.float32)
    for i in range(8):
        b, hh = i // 2, i % 2
        out[b, :, hh * HC:(hh + 1) * HC, :] = res.results[i]["outs"]
    return out
